# revision 14
# baseline (speedup 1.0000x reference)
"""Multi-head attention (B=2, S=2048, D=1024, H=16, dh=64) on 8 TRN2 NeuronCores.

Sharding: tensor-parallel over heads — 2 heads per core. Each core computes
Q/K/V projections for its 2 heads, full attention over S=2048, and a partial
output projection (its 128 rows of Wo). Host sums the 8 partial outputs + bo.

Per-core dataflow (matmuls in float32r; concurrent tile_position pairs):
  A) QKV^T:   psum[dh2=128, tok 512] = sum_k Wp_k[128,128].T @ xT_k[128,512]
  T) V^T -> V via PE transpose (ctx matmul needs t on partitions)
  B) scoresT: psum[t=128, s 512] = K^T_h[64,128].T @ Q^T_h[64,512] per head,
     row-tiled (h0 rows 0-63, h1 rows 64-127) -> concurrent pair
  E) expT = exp(0.125 * scoresT)  (ACT, scale folded; no max-subtraction —
     scores are O(1) by construction)
  C) ctx^T:   psum[128, 512]: h0 cols 0-63, h1 cols 64-127, col-tiled pairs
  L) l_h[s] = sum_t expT: 4 concurrent ones-chains (h x t-parity, M=32 col
     tiles) -> psum partition rows 0/32/64/96
  N) l combine + reciprocal in [128,4] layout (all 128 DVE lanes), 1/l
     broadcast via DRAM round-trip, one tensor_mul: ctxn = ctx * rbb
  D) out[s 128, d 512] = ctxn[:,s128][128,128].T @ Wo[128,512]
"""

import numpy as np

import concourse.bacc as bacc
import concourse.bass as bass
import concourse.mybir as mybir
import concourse.tile as tile
from concourse.bass_utils import run_bass_kernel_spmd

F32 = mybir.dt.float32
F32R = mybir.dt.float32r

B, S, D, H, DH = 2, 2048, 1024, 16, 64
TOK = B * S          # 4096
DH2 = 2 * DH         # 128 (two heads per core)
NCORES = 8
SC = 512             # s-chunk
NSC = S // SC        # 4 s-chunks per batch
NT = S // 128        # 16 t-tiles per batch
NKT = D // 128       # 8 k-tiles of contraction
NCH = TOK // SC      # 8 token chunks for stage A


def build_bass():
    nc = bacc.Bacc(None, target_bir_lowering=False)

    xT = nc.dram_tensor("xT", [D, TOK], F32, kind="ExternalInput")
    wq = nc.dram_tensor("wq", [D, DH2], F32, kind="ExternalInput")
    wk = nc.dram_tensor("wk", [D, DH2], F32, kind="ExternalInput")
    wv = nc.dram_tensor("wv", [D, DH2], F32, kind="ExternalInput")
    bqkv = nc.dram_tensor("bqkv", [3, DH2], F32, kind="ExternalInput")
    wo = nc.dram_tensor("wo", [DH2, D], F32, kind="ExternalInput")
    ones = nc.dram_tensor("ones", [128, 32], F32, kind="ExternalInput")
    iden = nc.dram_tensor("iden", [128, 128], F32, kind="ExternalInput")
    out = nc.dram_tensor("out", [TOK, D], F32, kind="ExternalOutput")

    with tile.TileContext(nc) as tc:
        with (
            tc.tile_pool(name="persist", bufs=1) as persist,
            tc.tile_pool(name="xin", bufs=10) as xin,
            tc.tile_pool(name="exps", bufs=4) as exps,
            tc.tile_pool(name="work", bufs=2) as work,
            tc.tile_pool(name="ost", bufs=2) as ost,
            tc.tile_pool(name="ps_big", bufs=2, space="PSUM") as ps_big,
            tc.tile_pool(name="ps_ctx", bufs=2, space="PSUM") as ps_ctx,
            tc.tile_pool(name="ps_out", bufs=2, space="PSUM") as ps_out,
            tc.tile_pool(name="dscratch", bufs=4, space="DRAM") as dscratch,
        ):
            # ---- constants / persistent tiles ----
            w_sb = persist.tile([128, 3, NKT, DH2], F32R, tag="w")
            for i, w in enumerate((wq, wk, wv)):
                nc.sync.dma_start(
                    out=w_sb[:, i, :, :],
                    in_=w.rearrange("(t p) m -> p t m", p=128).bitcast(F32R),
                )
            b_sb = persist.tile([128, 3], F32, tag="b")
            nc.gpsimd.dma_start(out=b_sb[:], in_=bqkv.rearrange("q p -> p q"))
            wo_sb = persist.tile([128, D], F32R, tag="wo")
            nc.sync.dma_start(out=wo_sb[:], in_=wo[:, :].bitcast(F32R))
            ident = persist.tile([128, 128], F32R, tag="id")
            nc.sync.dma_start(out=ident[:], in_=iden[:, :].bitcast(F32R))
            ones_sb = persist.tile([128, 32], F32R, tag="ones")
            nc.sync.dma_start(out=ones_sb[:], in_=ones[:, :].bitcast(F32R))

            qT = persist.tile([128, TOK], F32R, tag="qT")
            kT = persist.tile([128, TOK], F32R, tag="kT")
            vT = persist.tile([128, TOK], F32R, tag="vT")
            v_sb = persist.tile([128, TOK // 128, 130], F32R, tag="v")
            o1 = ones[0:1, 0:TOK // 128]
            ones_bc = bass.AP(tensor=o1.tensor, offset=o1.offset,
                              ap=[[0, 128], [1, TOK // 128]]).bitcast(F32R)
            nc.gpsimd.dma_start(out=v_sb[:, :, 64], in_=ones_bc)
            nc.gpsimd.dma_start(out=v_sb[:, :, 129], in_=ones_bc)

            xTv = xT.rearrange("(t p) n -> p t n", p=128)

            # ---- stage A: QKV projections ----
            for ch in range(NCH):
                c0 = ch * SC
                xts = []
                for kt in range(NKT):
                    x_t = xin.tile([128, SC], F32R, tag="x")
                    eng = nc.sync if kt % 2 == 0 else nc.gpsimd
                    eng.dma_start(out=x_t[:], in_=xTv[:, kt, c0:c0 + SC].bitcast(F32R))
                    xts.append(x_t)
                ps_qk = ps_big.tile([128, 1024], F32, tag="big")
                ps_v = ps_big.tile([128, 1024], F32, tag="big")
                dests = (qT, kT, vT)
                outs = (ps_qk[:, 0:512], ps_qk[:, 512:1024], ps_v[:, 0:512])
                for p in range(3):
                    for kt in range(NKT):
                        nc.tensor.matmul(
                            outs[p],
                            w_sb[:, p, kt, :],
                            xts[kt][:],
                            start=(kt == 0), stop=(kt == NKT - 1),
                        )
                for p in range(3):
                    nc.vector.tensor_scalar_add(
                        dests[p][:, c0:c0 + SC], outs[p], b_sb[:, p:p + 1]
                    )

            # ---- stage T: transpose V^T -> v_sb ----
            for blk in range(TOK // 128):
                ps_t = ps_ctx.tile([128, 128], F32R, tag="ctx")
                nc.tensor.transpose(ps_t[:], vT[:, blk * 128:(blk + 1) * 128], ident[:])
                nc.vector.tensor_copy(v_sb[:, blk, 0:64], ps_t[:, 0:64])
                nc.vector.tensor_copy(v_sb[:, blk, 65:129], ps_t[:, 64:128])

            # ---- main loop over (batch, s-chunk) ----
            for b in range(B):
                for sc in range(NSC):
                    q0 = b * S + sc * SC
                    # stage B + E: scores^T and exp, per t-tile
                    etiles = []
                    for tq in range(4):  # quarters of t-tiles for dep granularity
                        e_t = exps.tile([128, 4, 1024], F32R, tag="e")
                        etiles.append(e_t)
                    for tt in range(NT):
                        t0 = b * S + tt * 128
                        ps_s = ps_big.tile([128, 1024], F32, tag="big")
                        nc.tensor.matmul(
                            ps_s[:, 0:512],
                            kT[0:64, t0:t0 + 128],
                            qT[0:64, q0:q0 + SC],
                            start=True, stop=True,
                        )
                        nc.tensor.matmul(
                            ps_s[:, 512:1024],
                            kT[64:128, t0:t0 + 128],
                            qT[64:128, q0:q0 + SC],
                            start=True, stop=True,
                        )
                        nc.scalar.activation(
                            etiles[tt // 4][:, tt % 4, :], ps_s[:],
                            mybir.ActivationFunctionType.Exp, scale=0.125,
                        )

                    def erhs(tt, h):
                        return etiles[tt // 4][:, tt % 4, h * 512:h * 512 + 512]

                    # stage C: ctx^T aug per head (M=65, l in row 64)
                    ctxn = work.tile([128, SC], F32R, tag="ctxn")
                    for h in range(2):
                        ps_c = ps_ctx.tile([65, SC], F32, tag="ctx")
                        for tt in range(NT):
                            nc.tensor.matmul(
                                ps_c[:],
                                v_sb[:, b * NT + tt, h * 65:h * 65 + 65],
                                erhs(tt, h),
                                start=(tt == 0), stop=(tt == NT - 1),
                            )
                        # stage N: l -> DRAM -> [128,4] recip -> broadcast 1/l
                        lrow = work.tile([1, SC], F32, tag="lrow")
                        nc.vector.tensor_copy(lrow[:], ps_c[64:65, :])
                        ld = dscratch.tile([1, SC], F32, tag="ld")
                        nc.sync.dma_start(out=ld[:], in_=lrow[:])
                        ldsl = ld[:]
                        ls = work.tile([128, 4], F32, tag="ls")
                        nc.sync.dma_start(
                            out=ls[:],
                            in_=bass.AP(tensor=ldsl.tensor, offset=ldsl.offset,
                                        ap=[[1, 128], [128, 4]]),
                        )
                        rs = work.tile([128, 4], F32, tag="rs")
                        nc.vector.reciprocal(rs[:], ls[:])
                        rd = dscratch.tile([1, SC], F32, tag="rd")
                        rdsl = rd[:]
                        nc.sync.dma_start(
                            out=bass.AP(tensor=rdsl.tensor, offset=rdsl.offset,
                                        ap=[[1, 128], [128, 4]]),
                            in_=rs[:],
                        )
                        rbb = work.tile([64, SC], F32, tag="rbb")
                        nc.gpsimd.dma_start(
                            out=rbb[:],
                            in_=bass.AP(tensor=rdsl.tensor, offset=rdsl.offset,
                                        ap=[[0, 64], [1, SC]]),
                        )
                        if h == 0:
                            nc.vector.tensor_mul(ctxn[0:64, :], ps_c[0:64, :], rbb[:])
                        else:
                            tmp = work.tile([64, SC], F32R, tag="tmp")
                            nc.vector.tensor_mul(tmp[:], ps_c[0:64, :], rbb[:])
                            nc.sync.dma_start(out=ctxn[64:128, :], in_=tmp[:])
                    # stage D: output projection (partial)
                    for ss in range(SC // 128):
                        o_sb = ost.tile([128, 1024], F32, tag="o")
                        for dc in range(2):
                            ps_o = ps_out.tile([128, 512], F32, tag="out")
                            nc.tensor.matmul(
                                ps_o[:],
                                ctxn[:, ss * 128:(ss + 1) * 128],
                                wo_sb[:, dc * 512:(dc + 1) * 512],
                                start=True, stop=True,
                            )
                            nc.vector.tensor_copy(o_sb[:, dc * 512:(dc + 1) * 512], ps_o[:])
                        nc.gpsimd.dma_start(
                            out=out[q0 + ss * 128:q0 + (ss + 1) * 128, :], in_=o_sb[:]
                        )
    nc.finalize()
    return nc


_NC_CACHE = None


def make_in_maps(x, Wq, Wk, Wv, bq, bk, bv, Wo, bo=None):
    xT = np.ascontiguousarray(x.reshape(TOK, D).T)  # [D, TOK]
    in_maps = []
    for c in range(NCORES):
        h0 = 2 * c
        in_maps.append({
            "xT": xT,
            "wq": np.ascontiguousarray(np.concatenate([Wq[h0], Wq[h0 + 1]], axis=1)),
            "wk": np.ascontiguousarray(np.concatenate([Wk[h0], Wk[h0 + 1]], axis=1)),
            "wv": np.ascontiguousarray(np.concatenate([Wv[h0], Wv[h0 + 1]], axis=1)),
            "bqkv": np.ascontiguousarray(np.stack([
                bq[h0:h0 + 2].reshape(DH2),
                bk[h0:h0 + 2].reshape(DH2),
                bv[h0:h0 + 2].reshape(DH2),
            ])),
            "wo": np.ascontiguousarray(Wo[c * DH2:(c + 1) * DH2]),
            "ones": np.ones((128, 32), dtype=np.float32),
            "iden": np.eye(128, dtype=np.float32),
        })
    return in_maps


def kernel(x, Wq, Wk, Wv, bq, bk, bv, Wo, bo):
    global _NC_CACHE
    if _NC_CACHE is None:
        _NC_CACHE = build_bass()
    nc = _NC_CACHE

    in_maps = make_in_maps(x, Wq, Wk, Wv, bq, bk, bv, Wo)
    res = run_bass_kernel_spmd(nc, in_maps, list(range(NCORES)))
    acc = np.zeros((TOK, D), dtype=np.float64)
    for c in range(NCORES):
        acc += res.results[c]["out"]
    acc += bo
    return acc.astype(np.float32).reshape(B, S, D)


# revision 17
# speedup vs baseline: 1.0541x; 1.0541x over previous
"""Multi-head attention (B=2, S=2048, D=1024, H=16, dh=64) on 8 TRN2 NeuronCores.

Sharding: tensor-parallel over heads — 2 heads per core. Each core computes
Q/K/V projections for its 2 heads, full attention over S=2048, and a partial
output projection (its 128 rows of Wo). Host sums the 8 partial outputs + bo.

Per-core dataflow (matmuls in float32r; concurrent tile_position pairs):
  A) QKV^T:   psum[dh2=128, tok 512] = sum_k Wp_k[128,128].T @ xT_k[128,512]
  T) V^T -> V via PE transpose (ctx matmul needs t on partitions)
  B) scoresT: psum[t=128, s 512] = K^T_h[64,128].T @ Q^T_h[64,512] per head,
     row-tiled (h0 rows 0-63, h1 rows 64-127) -> concurrent pair
  E) expT = exp(0.125 * scoresT)  (ACT, scale folded; no max-subtraction —
     scores are O(1) by construction)
  C) ctx^T:   psum[128, 512]: h0 cols 0-63, h1 cols 64-127, col-tiled pairs
  L) l_h[s] = sum_t expT: 4 concurrent ones-chains (h x t-parity, M=32 col
     tiles) -> psum partition rows 0/32/64/96
  N) l combine + reciprocal in [128,4] layout (all 128 DVE lanes), 1/l
     broadcast via DRAM round-trip, one tensor_mul: ctxn = ctx * rbb
  D) out[s 128, d 512] = ctxn[:,s128][128,128].T @ Wo[128,512]
"""

import numpy as np

import concourse.bacc as bacc
import concourse.bass as bass
import concourse.mybir as mybir
import concourse.tile as tile
from concourse.bass_utils import run_bass_kernel_spmd

F32 = mybir.dt.float32
F32R = mybir.dt.float32r

B, S, D, H, DH = 2, 2048, 1024, 16, 64
TOK = B * S          # 4096
DH2 = 2 * DH         # 128 (two heads per core)
NCORES = 8
SC = 512             # s-chunk
NSC = S // SC        # 4 s-chunks per batch
NT = S // 128        # 16 t-tiles per batch
NKT = D // 128       # 8 k-tiles of contraction
NCH = TOK // SC      # 8 token chunks for stage A


def build_bass():
    nc = bacc.Bacc(None, target_bir_lowering=False)

    xT = nc.dram_tensor("xT", [D, TOK], F32, kind="ExternalInput")
    wq = nc.dram_tensor("wq", [D, DH2], F32, kind="ExternalInput")
    wk = nc.dram_tensor("wk", [D, DH2], F32, kind="ExternalInput")
    wv = nc.dram_tensor("wv", [D, DH2], F32, kind="ExternalInput")
    bqkv = nc.dram_tensor("bqkv", [3, DH2], F32, kind="ExternalInput")
    wo = nc.dram_tensor("wo", [DH2, D], F32, kind="ExternalInput")
    ones = nc.dram_tensor("ones", [128, 32], F32, kind="ExternalInput")
    iden = nc.dram_tensor("iden", [128, 128], F32, kind="ExternalInput")
    out = nc.dram_tensor("out", [TOK, D], F32, kind="ExternalOutput")

    with tile.TileContext(nc) as tc:
        with (
            tc.tile_pool(name="persist", bufs=1) as persist,
            tc.tile_pool(name="xin", bufs=10) as xin,
            tc.tile_pool(name="exps", bufs=8) as exps,
            tc.tile_pool(name="work", bufs=2) as work,
            tc.tile_pool(name="ost", bufs=2) as ost,
            tc.tile_pool(name="ps_big", bufs=2, space="PSUM") as ps_big,
            tc.tile_pool(name="ps_ctx", bufs=2, space="PSUM") as ps_ctx,
            tc.tile_pool(name="ps_out", bufs=2, space="PSUM") as ps_out,
            tc.tile_pool(name="dscratch", bufs=4, space="DRAM") as dscratch,
        ):
            # ---- constants / persistent tiles ----
            w_sb = persist.tile([128, 3, NKT, DH2], F32R, tag="w")
            for i, w in enumerate((wq, wk, wv)):
                nc.sync.dma_start(
                    out=w_sb[:, i, :, :],
                    in_=w.rearrange("(t p) m -> p t m", p=128).bitcast(F32R),
                )
            b_sb = persist.tile([128, 3], F32, tag="b")
            nc.gpsimd.dma_start(out=b_sb[:], in_=bqkv.rearrange("q p -> p q"))
            wo_sb = persist.tile([128, D], F32R, tag="wo")
            nc.sync.dma_start(out=wo_sb[:], in_=wo[:, :].bitcast(F32R))
            ident = persist.tile([128, 128], F32R, tag="id")
            nc.sync.dma_start(out=ident[:], in_=iden[:, :].bitcast(F32R))
            ones_sb = persist.tile([128, 32], F32R, tag="ones")
            nc.sync.dma_start(out=ones_sb[:], in_=ones[:, :].bitcast(F32R))

            qT = persist.tile([128, TOK], F32R, tag="qT")
            kT = persist.tile([128, TOK], F32R, tag="kT")
            vT = persist.tile([128, TOK], F32R, tag="vT")
            v_sb = persist.tile([128, TOK // 128, 130], F32R, tag="v")
            o1 = ones[0:1, 0:TOK // 128]
            ones_bc = bass.AP(tensor=o1.tensor, offset=o1.offset,
                              ap=[[0, 128], [1, TOK // 128]]).bitcast(F32R)
            nc.gpsimd.dma_start(out=v_sb[:, :, 64], in_=ones_bc)
            nc.gpsimd.dma_start(out=v_sb[:, :, 129], in_=ones_bc)

            xTv = xT.rearrange("(t p) n -> p t n", p=128)

            # ---- stage A: QKV projections ----
            for ch in range(NCH):
                c0 = ch * SC
                xts = []
                for kt in range(NKT):
                    x_t = xin.tile([128, SC], F32R, tag="x")
                    eng = nc.sync if kt % 2 == 0 else nc.gpsimd
                    eng.dma_start(out=x_t[:], in_=xTv[:, kt, c0:c0 + SC].bitcast(F32R))
                    xts.append(x_t)
                ps_qk = ps_big.tile([128, 1024], F32, tag="big")
                ps_v = ps_big.tile([128, 1024], F32, tag="big")
                dests = (qT, kT, vT)
                outs = (ps_qk[:, 0:512], ps_qk[:, 512:1024], ps_v[:, 0:512])
                for p in range(3):
                    for kt in range(NKT):
                        nc.tensor.matmul(
                            outs[p],
                            w_sb[:, p, kt, :],
                            xts[kt][:],
                            start=(kt == 0), stop=(kt == NKT - 1),
                        )
                for p in range(3):
                    nc.vector.tensor_scalar_add(
                        dests[p][:, c0:c0 + SC], outs[p], b_sb[:, p:p + 1]
                    )

            # ---- stage T: transpose V^T -> v_sb ----
            for blk in range(TOK // 128):
                ps_t = ps_ctx.tile([128, 128], F32R, tag="ctx")
                nc.tensor.transpose(ps_t[:], vT[:, blk * 128:(blk + 1) * 128], ident[:])
                nc.vector.tensor_copy(v_sb[:, blk, 0:64], ps_t[:, 0:64])
                nc.vector.tensor_copy(v_sb[:, blk, 65:129], ps_t[:, 64:128])

            # ---- main loop over (batch, s-chunk) ----
            for b in range(B):
                for sc in range(NSC):
                    q0 = b * S + sc * SC
                    # stage B + E: scores^T and exp, per t-tile
                    etiles = []
                    for tq in range(8):  # octets of t-tiles for dep granularity
                        e_t = exps.tile([128, 2, 1024], F32R, tag="e")
                        etiles.append(e_t)
                    for tt in range(NT):
                        t0 = b * S + tt * 128
                        ps_s = ps_big.tile([128, 1024], F32, tag="big")
                        nc.tensor.matmul(
                            ps_s[:, 0:512],
                            kT[0:64, t0:t0 + 128],
                            qT[0:64, q0:q0 + SC],
                            start=True, stop=True,
                        )
                        nc.tensor.matmul(
                            ps_s[:, 512:1024],
                            kT[64:128, t0:t0 + 128],
                            qT[64:128, q0:q0 + SC],
                            start=True, stop=True,
                        )
                        nc.scalar.activation(
                            etiles[tt // 2][:, tt % 2, :], ps_s[:],
                            mybir.ActivationFunctionType.Exp, scale=0.125,
                        )

                    def erhs(tt, h):
                        return etiles[tt // 2][:, tt % 2, h * 512:h * 512 + 512]

                    # stage C: ctx^T aug, heads interleaved (frees expT early)
                    ctxn = work.tile([128, SC], F32R, tag="ctxn")
                    pcs = []
                    for h in range(2):
                        pc_h = ps_ctx.tile([65, SC], F32, tag="ctx")
                        pcs.append(pc_h)
                    for tt in range(NT):
                        for h in range(2):
                            nc.tensor.matmul(
                                pcs[h][:],
                                v_sb[:, b * NT + tt, h * 65:h * 65 + 65],
                                erhs(tt, h),
                                start=(tt == 0), stop=(tt == NT - 1),
                            )
                    for h in range(2):
                        ps_c = pcs[h]
                        # norm: l row -> [128,4] recip -> DRAM -> broadcast 1/l
                        lrow = work.tile([1, SC], F32, tag="lrow")
                        nc.vector.tensor_copy(lrow[:], ps_c[64:65, :])
                        ld = dscratch.tile([1, SC], F32, tag="ld")
                        nc.sync.dma_start(out=ld[:], in_=lrow[:])
                        ldsl = ld[:]
                        ls = work.tile([128, 4], F32, tag="ls")
                        nc.sync.dma_start(
                            out=ls[:],
                            in_=bass.AP(tensor=ldsl.tensor, offset=ldsl.offset,
                                        ap=[[1, 128], [128, 4]]),
                        )
                        rs = work.tile([128, 4], F32, tag="rs")
                        nc.vector.reciprocal(rs[:], ls[:])
                        rd = dscratch.tile([1, SC], F32, tag="rd")
                        rdsl = rd[:]
                        nc.sync.dma_start(
                            out=bass.AP(tensor=rdsl.tensor, offset=rdsl.offset,
                                        ap=[[1, 128], [128, 4]]),
                            in_=rs[:],
                        )
                        rbb = work.tile([64, SC], F32, tag="rbb")
                        nc.gpsimd.dma_start(
                            out=rbb[:],
                            in_=bass.AP(tensor=rdsl.tensor, offset=rdsl.offset,
                                        ap=[[0, 64], [1, SC]]),
                        )
                        if h == 0:
                            nc.vector.tensor_mul(ctxn[0:64, :], ps_c[0:64, :], rbb[:])
                        else:
                            tmp = work.tile([64, SC], F32R, tag="tmp")
                            nc.vector.tensor_mul(tmp[:], ps_c[0:64, :], rbb[:])
                            nc.gpsimd.dma_start(out=ctxn[64:128, :], in_=tmp[:])
                    # stage D: output projection (partial)
                    for ss in range(SC // 128):
                        o_sb = ost.tile([128, 1024], F32, tag="o")
                        for dc in range(2):
                            ps_o = ps_out.tile([128, 512], F32, tag="out")
                            nc.tensor.matmul(
                                ps_o[:],
                                ctxn[:, ss * 128:(ss + 1) * 128],
                                wo_sb[:, dc * 512:(dc + 1) * 512],
                                start=True, stop=True,
                            )
                            nc.vector.tensor_copy(o_sb[:, dc * 512:(dc + 1) * 512], ps_o[:])
                        nc.gpsimd.dma_start(
                            out=out[q0 + ss * 128:q0 + (ss + 1) * 128, :], in_=o_sb[:]
                        )
    nc.finalize()
    return nc


_NC_CACHE = None


def make_in_maps(x, Wq, Wk, Wv, bq, bk, bv, Wo, bo=None):
    xT = np.ascontiguousarray(x.reshape(TOK, D).T)  # [D, TOK]
    in_maps = []
    for c in range(NCORES):
        h0 = 2 * c
        in_maps.append({
            "xT": xT,
            "wq": np.ascontiguousarray(np.concatenate([Wq[h0], Wq[h0 + 1]], axis=1)),
            "wk": np.ascontiguousarray(np.concatenate([Wk[h0], Wk[h0 + 1]], axis=1)),
            "wv": np.ascontiguousarray(np.concatenate([Wv[h0], Wv[h0 + 1]], axis=1)),
            "bqkv": np.ascontiguousarray(np.stack([
                bq[h0:h0 + 2].reshape(DH2),
                bk[h0:h0 + 2].reshape(DH2),
                bv[h0:h0 + 2].reshape(DH2),
            ])),
            "wo": np.ascontiguousarray(Wo[c * DH2:(c + 1) * DH2]),
            "ones": np.ones((128, 32), dtype=np.float32),
            "iden": np.eye(128, dtype=np.float32),
        })
    return in_maps


def kernel(x, Wq, Wk, Wv, bq, bk, bv, Wo, bo):
    global _NC_CACHE
    if _NC_CACHE is None:
        _NC_CACHE = build_bass()
    nc = _NC_CACHE

    in_maps = make_in_maps(x, Wq, Wk, Wv, bq, bk, bv, Wo)
    res = run_bass_kernel_spmd(nc, in_maps, list(range(NCORES)))
    acc = np.zeros((TOK, D), dtype=np.float64)
    for c in range(NCORES):
        acc += res.results[c]["out"]
    acc += bo
    return acc.astype(np.float32).reshape(B, S, D)


# revision 18
# speedup vs baseline: 1.2481x; 1.1841x over previous
"""Multi-head attention (B=2, S=2048, D=1024, H=16, dh=64) on 8 TRN2 NeuronCores.

Sharding: tensor-parallel over heads — 2 heads per core. Each core computes
Q/K/V projections for its 2 heads, full attention over S=2048, and a partial
output projection (its 128 rows of Wo). Host sums the 8 partial outputs + bo.

Per-core dataflow (all matmuls in float32r, 1 cyc/row at N=512):
  A) QKV^T:   psum[dh2=128, tok 512] = sum_k Wp_k[128,128].T @ xT_k[128,512]
  T) V^T -> V via PE transpose (ctx matmul needs t on partitions)
  B) scoresT: psum[t=128, s 512] = K^T_h[64,128].T @ Q^T_h[64,512]  (2 heads
     row-tiled into one [128,1024] psum tile)
  E) expT = exp(0.125 * scoresT)  (ACT, scale folded; no max-subtraction —
     scores are O(1) by construction)
  C) ctx^T aug: psum[65, 512] = sum_t [V_h|1][128,65].T @ expT[128,512]
     row 64 = softmax denominator l
  N) ctxn = ctx * (1/l)  (DVE recip + DMA partition-broadcast + DVE mult)
  D) out[s 128, d 512] = ctxn[:,s128][128,128].T @ Wo[128,512]
"""

import numpy as np

import concourse.bacc as bacc
import concourse.mybir as mybir
import concourse.tile as tile
from concourse.bass_utils import run_bass_kernel_spmd

F32 = mybir.dt.float32
F32R = mybir.dt.float32r

B, S, D, H, DH = 2, 2048, 1024, 16, 64
TOK = B * S          # 4096
DH2 = 2 * DH         # 128 (two heads per core)
NCORES = 8
SC = 512             # s-chunk
NSC = S // SC        # 4 s-chunks per batch
NT = S // 128        # 16 t-tiles per batch
NKT = D // 128       # 8 k-tiles of contraction
NCH = TOK // SC      # 8 token chunks for stage A


def build_bass():
    nc = bacc.Bacc(None, target_bir_lowering=False)

    xT = nc.dram_tensor("xT", [D, TOK], F32, kind="ExternalInput")
    wq = nc.dram_tensor("wq", [D, DH2], F32, kind="ExternalInput")
    wk = nc.dram_tensor("wk", [D, DH2], F32, kind="ExternalInput")
    wv = nc.dram_tensor("wv", [D, DH2], F32, kind="ExternalInput")
    bqkv = nc.dram_tensor("bqkv", [3, DH2], F32, kind="ExternalInput")
    wo = nc.dram_tensor("wo", [DH2, D], F32, kind="ExternalInput")
    ones = nc.dram_tensor("ones", [128, 32], F32, kind="ExternalInput")
    iden = nc.dram_tensor("iden", [128, 128], F32, kind="ExternalInput")
    out = nc.dram_tensor("out", [TOK, D], F32, kind="ExternalOutput")

    with tile.TileContext(nc) as tc:
        with (
            tc.tile_pool(name="persist", bufs=1) as persist,
            tc.tile_pool(name="xin", bufs=10) as xin,
            tc.tile_pool(name="exps", bufs=8) as exps,
            tc.tile_pool(name="work", bufs=2) as work,
            tc.tile_pool(name="ost", bufs=2) as ost,
            tc.tile_pool(name="ps_big", bufs=2, space="PSUM") as ps_big,
            tc.tile_pool(name="ps_ctx", bufs=2, space="PSUM") as ps_ctx,
            tc.tile_pool(name="ps_out", bufs=2, space="PSUM") as ps_out,
            tc.tile_pool(name="dscratch", bufs=2, space="DRAM") as dscratch,
        ):
            # ---- constants / persistent tiles ----
            w_sb = persist.tile([128, 3, NKT, DH2], F32R, tag="w")
            for i, w in enumerate((wq, wk, wv)):
                nc.sync.dma_start(
                    out=w_sb[:, i, :, :],
                    in_=w.rearrange("(t p) m -> p t m", p=128).bitcast(F32R),
                )
            b_sb = persist.tile([128, 3], F32, tag="b")
            nc.gpsimd.dma_start(out=b_sb[:], in_=bqkv.rearrange("q p -> p q"))
            wo_sb = persist.tile([128, D], F32R, tag="wo")
            nc.sync.dma_start(out=wo_sb[:], in_=wo[:, :].bitcast(F32R))
            ident = persist.tile([128, 128], F32R, tag="id")
            nc.sync.dma_start(out=ident[:], in_=iden[:, :].bitcast(F32R))

            qT = persist.tile([128, TOK], F32R, tag="qT")
            kT = persist.tile([128, TOK], F32R, tag="kT")
            vT = persist.tile([128, TOK], F32R, tag="vT")
            # V in [t, e] layout, 130 = [V_h0(64) | 1 | V_h1(64) | 1]
            v_sb = persist.tile([128, TOK // 128, 130], F32R, tag="v")
            import concourse.bass as bass_mod
            o1 = ones[0:1, 0:TOK // 128]
            ones_b = bass_mod.AP(tensor=o1.tensor, offset=o1.offset,
                                 ap=[[0, 128], [1, TOK // 128]]).bitcast(F32R)
            nc.gpsimd.dma_start(out=v_sb[:, :, 64], in_=ones_b)
            nc.gpsimd.dma_start(out=v_sb[:, :, 129], in_=ones_b)

            xTv = xT.rearrange("(t p) n -> p t n", p=128)

            # ---- stage A: QKV projections ----
            for ch in range(NCH):
                c0 = ch * SC
                xts = []
                for kt in range(NKT):
                    x_t = xin.tile([128, SC], F32R, tag="x")
                    eng = nc.sync if kt % 2 == 0 else nc.gpsimd
                    eng.dma_start(out=x_t[:], in_=xTv[:, kt, c0:c0 + SC].bitcast(F32R))
                    xts.append(x_t)
                ps_qk = ps_big.tile([128, 1024], F32, tag="big")
                ps_v = ps_big.tile([128, 1024], F32, tag="big")
                dests = (qT, kT, vT)
                outs = (ps_qk[:, 0:512], ps_qk[:, 512:1024], ps_v[:, 0:512])
                for p in range(3):
                    for kt in range(NKT):
                        nc.tensor.matmul(
                            outs[p],
                            w_sb[:, p, kt, :],
                            xts[kt][:],
                            start=(kt == 0), stop=(kt == NKT - 1),
                        )
                for p in range(3):
                    nc.vector.tensor_scalar_add(
                        dests[p][:, c0:c0 + SC], outs[p], b_sb[:, p:p + 1]
                    )

            # ---- stage T: transpose V^T -> v_sb ----
            for blk in range(TOK // 128):
                ps_t = ps_ctx.tile([128, 128], F32R, tag="ctx")
                nc.tensor.transpose(ps_t[:], vT[:, blk * 128:(blk + 1) * 128], ident[:])
                nc.vector.tensor_copy(v_sb[:, blk, 0:64], ps_t[:, 0:64])
                nc.vector.tensor_copy(v_sb[:, blk, 65:129], ps_t[:, 64:128])

            # ---- main loop over (batch, s-chunk) ----
            for b in range(B):
                for sc in range(NSC):
                    q0 = b * S + sc * SC
                    # stage B + E: scores^T and exp, per t-tile
                    etiles = []
                    for tq in range(8):  # octets of t-tiles for dep granularity
                        e_t = exps.tile([128, 2, 1024], F32R, tag="e")
                        etiles.append(e_t)
                    for tt in range(NT):
                        t0 = b * S + tt * 128
                        ps_s = ps_big.tile([128, 1024], F32, tag="big")
                        nc.tensor.matmul(
                            ps_s[:, 0:512],
                            kT[0:64, t0:t0 + 128],
                            qT[0:64, q0:q0 + SC],
                            start=True, stop=True,
                        )
                        nc.tensor.matmul(
                            ps_s[:, 512:1024],
                            kT[64:128, t0:t0 + 128],
                            qT[64:128, q0:q0 + SC],
                            start=True, stop=True,
                        )
                        nc.scalar.activation(
                            etiles[tt // 2][:, tt % 2, :], ps_s[:],
                            mybir.ActivationFunctionType.Exp, scale=0.125,
                        )
                    # stage C + N: ctx, denominators, normalize
                    ctxn = work.tile([128, SC], F32R, tag="ctxn")
                    for h in range(2):
                        ps_c = ps_ctx.tile([65, SC], F32, tag="ctx")
                        for tt in range(NT):
                            nc.tensor.matmul(
                                ps_c[:],
                                v_sb[:, b * NT + tt, h * 65:h * 65 + 65],
                                etiles[tt // 2][:, tt % 2, h * 512:h * 512 + 512],
                                start=(tt == 0), stop=(tt == NT - 1),
                            )
                        rb = work.tile([1, SC], F32, tag="rb")
                        nc.vector.reciprocal(rb[:], ps_c[64:65, :])
                        rd = dscratch.tile([1, SC], F32, tag="rd")
                        nc.sync.dma_start(out=rd[:], in_=rb[:])
                        rbb = work.tile([64, SC], F32, tag="rbb")
                        nc.gpsimd.dma_start(out=rbb[:], in_=rd[:].to_broadcast([64, SC]))
                        if h == 0:
                            nc.vector.tensor_mul(ctxn[0:64, :], ps_c[0:64, :], rbb[:])
                        else:
                            tmp = work.tile([64, SC], F32R, tag="tmp")
                            nc.vector.tensor_mul(tmp[:], ps_c[0:64, :], rbb[:])
                            nc.gpsimd.dma_start(out=ctxn[64:128, :], in_=tmp[:])
                    # stage D: output projection (partial)
                    for ss in range(SC // 128):
                        o_sb = ost.tile([128, 1024], F32, tag="o")
                        for dc in range(2):
                            ps_o = ps_out.tile([128, 512], F32, tag="out")
                            nc.tensor.matmul(
                                ps_o[:],
                                ctxn[:, ss * 128:(ss + 1) * 128],
                                wo_sb[:, dc * 512:(dc + 1) * 512],
                                start=True, stop=True,
                            )
                            nc.vector.tensor_copy(o_sb[:, dc * 512:(dc + 1) * 512], ps_o[:])
                        nc.gpsimd.dma_start(
                            out=out[q0 + ss * 128:q0 + (ss + 1) * 128, :], in_=o_sb[:]
                        )
    nc.finalize()
    return nc


_NC_CACHE = None


def make_in_maps(x, Wq, Wk, Wv, bq, bk, bv, Wo, bo=None):
    xT = np.ascontiguousarray(x.reshape(TOK, D).T)  # [D, TOK]
    in_maps = []
    for c in range(NCORES):
        h0 = 2 * c
        in_maps.append({
            "xT": xT,
            "wq": np.ascontiguousarray(np.concatenate([Wq[h0], Wq[h0 + 1]], axis=1)),
            "wk": np.ascontiguousarray(np.concatenate([Wk[h0], Wk[h0 + 1]], axis=1)),
            "wv": np.ascontiguousarray(np.concatenate([Wv[h0], Wv[h0 + 1]], axis=1)),
            "bqkv": np.ascontiguousarray(np.stack([
                bq[h0:h0 + 2].reshape(DH2),
                bk[h0:h0 + 2].reshape(DH2),
                bv[h0:h0 + 2].reshape(DH2),
            ])),
            "wo": np.ascontiguousarray(Wo[c * DH2:(c + 1) * DH2]),
            "ones": np.ones((128, 32), dtype=np.float32),
            "iden": np.eye(128, dtype=np.float32),
        })
    return in_maps


def kernel(x, Wq, Wk, Wv, bq, bk, bv, Wo, bo):
    global _NC_CACHE
    if _NC_CACHE is None:
        _NC_CACHE = build_bass()
    nc = _NC_CACHE

    in_maps = make_in_maps(x, Wq, Wk, Wv, bq, bk, bv, Wo)
    res = run_bass_kernel_spmd(nc, in_maps, list(range(NCORES)))
    acc = np.zeros((TOK, D), dtype=np.float64)
    for c in range(NCORES):
        acc += res.results[c]["out"]
    acc += bo
    return acc.astype(np.float32).reshape(B, S, D)
